# revision 1
# baseline (speedup 1.0000x reference)
"""Trainium2 Bass kernel for nn_CategorySpecificInitNet (moe_routing).

kernel(**inputs) takes the FULL unsharded inputs (keys as in
reference.setup_inputs()) and returns the FULL [B, 128] float32 output.

Strategy (data-parallel, per spec sharding hint):
  - shard the batch B=32768 across 8 NeuronCores (4096 rows each);
  - replicate the encoder + all K=8 decoder weight stacks on every core;
  - on each core, run everything feature-major [features(partitions),
    rows(free)] so no transposes are ever needed on device (the host
    passes features pre-transposed);
  - the per-row category routing is done on-device: a one-hot mask is
    computed from cat_idx with iota/is_equal ops, decoder outputs are
    masked via fused scalar_tensor_tensor and summed across the K
    decoders directly inside the PSUM accumulation of the last matmul,
    with per-category output biases added through a tiny [8 x 128]
    matmul against the one-hot mask.

Matmuls run in bf16 (fp32 accumulation in PSUM).
"""
import sys

for _p in ("/opt/trn_rl_repo",):
    if _p not in sys.path:
        sys.path.append(_p)

import numpy as np
import ml_dtypes

import concourse.bass as bass
import concourse.bacc as bacc
import concourse.mybir as mybir
import concourse.tile as tile
from concourse import bass_utils

BF16 = mybir.dt.bfloat16
F32 = mybir.dt.float32
Alu = mybir.AluOpType
ActF = mybir.ActivationFunctionType

B, C, H1, H2, HO = 32768, 768, 512, 256, 256
DH, LAT, K = 256, 128, 8
N_CORES = 8
B_LOC = B // N_CORES
TILE = 512
# bias_all column offsets: be1[4] be2[2] be3[2] bd1[16] bd2[16] iota[1]
OB1, OB2, OB3, OD1, OD2, OIO = 0, 4, 6, 8, 24, 40
NBIAS = 41


def _build_nc(B_loc=B_LOC, tile_n=TILE, ps_w_bufs=6, dp_bufs=3):
    assert B_loc % tile_n == 0
    nt = B_loc // tile_n
    nc = bacc.Bacc(name="catnet")

    fT = nc.dram_tensor("fT", (C, B_loc), BF16, kind="ExternalInput")
    we1 = nc.dram_tensor("we1", (C, H1), BF16, kind="ExternalInput")
    we2 = nc.dram_tensor("we2", (H1, H2), BF16, kind="ExternalInput")
    we3 = nc.dram_tensor("we3", (H2, HO), BF16, kind="ExternalInput")
    wd1 = nc.dram_tensor("wd1", (K, HO, DH), BF16, kind="ExternalInput")
    wd2 = nc.dram_tensor("wd2", (K, DH, DH), BF16, kind="ExternalInput")
    wd3 = nc.dram_tensor("wd3", (K, DH, LAT), BF16, kind="ExternalInput")
    bias_all = nc.dram_tensor("bias_all", (128, NBIAS), F32, kind="ExternalInput")
    bd3w = nc.dram_tensor("bd3w", (K, LAT), BF16, kind="ExternalInput")
    cat = nc.dram_tensor("cat", (1, B_loc), BF16, kind="ExternalInput")
    ones = nc.dram_tensor("ones", (1, 128), BF16, kind="ExternalInput")
    out = nc.dram_tensor("out", (LAT, B_loc), F32, kind="ExternalOutput")

    nC, nH1, nH2, nHO, nDH = C // 128, H1 // 128, H2 // 128, HO // 128, DH // 128

    with tile.TileContext(nc) as tc:
        with (
            tc.tile_pool(name="wp", bufs=1) as wp,
            tc.tile_pool(name="fp", bufs=2) as fp,
            tc.tile_pool(name="ap", bufs=2) as ap,
            tc.tile_pool(name="dp", bufs=dp_bufs) as dp,
            tc.tile_pool(name="ps_w", bufs=ps_w_bufs, space="PSUM") as ps_w,
            tc.tile_pool(name="ps_o", bufs=2, space="PSUM") as ps_o,
        ):
            # small consts first, on the sync queue
            bias_t = wp.tile([128, NBIAS], F32, tag="bias")
            nc.sync.dma_start(bias_t[:], bias_all[:])
            bd3w_t = wp.tile([K, LAT], BF16, tag="bd3w")
            nc.sync.dma_start(bd3w_t[:], bd3w[:])
            ones_t = wp.tile([1, 128], BF16, tag="ones")
            nc.sync.dma_start(ones_t[:], ones[:])
            cat_t = wp.tile([1, B_loc], BF16, tag="cat")
            nc.sync.dma_start(cat_t[:], cat[:])

            # resident weights: one consolidated DMA per tensor, pool queue
            we1_t = wp.tile([128, nC, H1], BF16, tag="we1")
            nc.gpsimd.dma_start(we1_t[:], we1.rearrange("(c p) h -> p c h", p=128))
            we2_t = wp.tile([128, nH1, H2], BF16, tag="we2")
            nc.gpsimd.dma_start(we2_t[:], we2.rearrange("(c p) h -> p c h", p=128))
            we3_t = wp.tile([128, nH2, HO], BF16, tag="we3")
            nc.gpsimd.dma_start(we3_t[:], we3.rearrange("(c p) h -> p c h", p=128))
            wd1_t = wp.tile([128, K, nHO, DH], BF16, tag="wd1")
            nc.gpsimd.dma_start(wd1_t[:], wd1.rearrange("k (c p) d -> p k c d", p=128))
            wd2_t = wp.tile([128, K, nDH, DH], BF16, tag="wd2")
            nc.gpsimd.dma_start(wd2_t[:], wd2.rearrange("k (c p) d -> p k c d", p=128))
            wd3_t = wp.tile([128, K, nDH, LAT], BF16, tag="wd3")
            nc.gpsimd.dma_start(wd3_t[:], wd3.rearrange("k (c p) d -> p k c d", p=128))
            iota_ap = bias_t[0:K, OIO:OIO + 1]

            for t in range(nt):
                sl = bass.ts(t, tile_n)
                ftb = fp.tile([128, nC, tile_n], BF16, tag="ft")
                nc.sync.dma_start(
                    ftb[:], fT.rearrange("(c p) b -> p c b", p=128)[:, :, sl])
                ft = [ftb[:, c, :] for c in range(nC)]

                # category map broadcast to 128 partitions (PE outer product)
                pbc = ps_w.tile([128, tile_n], F32, tag="pw")
                nc.tensor.matmul(pbc[:], ones_t[:], cat_t[:, sl], start=True,
                                 stop=True)
                c_full = ap.tile([128, tile_n], BF16, tag="cfull")
                nc.scalar.activation(c_full[:], pbc[:], ActF.Copy)
                # one-hot mask rows [8, tile_n] for the bd3 matmul
                pb8 = ps_w.tile([8, tile_n], F32, tag="pw")
                nc.tensor.matmul(pb8[:], ones_t[:, :K], cat_t[:, sl], start=True,
                                 stop=True)
                mask8 = ap.tile([K, tile_n], BF16, tag="mask8")
                nc.vector.tensor_scalar(mask8[:], pb8[:], iota_ap, None, Alu.is_equal)

                # encoder
                a1 = []
                for m in range(nH1):
                    pw = ps_w.tile([128, tile_n], F32, tag="pw")
                    for c in range(nC):
                        nc.tensor.matmul(pw[:], we1_t[:, c, bass.ts(m, 128)], ft[c],
                                         start=(c == 0), stop=(c == nC - 1))
                    x = ap.tile([128, tile_n], BF16, tag=f"a1_{m}")
                    nc.scalar.activation(x[:], pw[:], ActF.Relu,
                                         bias=bias_t[:, OB1 + m:OB1 + m + 1])
                    a1.append(x)
                a2 = []
                for m in range(nH2):
                    pw = ps_w.tile([128, tile_n], F32, tag="pw")
                    for c in range(nH1):
                        nc.tensor.matmul(pw[:], we2_t[:, c, bass.ts(m, 128)], a1[c][:],
                                         start=(c == 0), stop=(c == nH1 - 1))
                    x = ap.tile([128, tile_n], BF16, tag=f"a2_{m}")
                    nc.scalar.activation(x[:], pw[:], ActF.Relu,
                                         bias=bias_t[:, OB2 + m:OB2 + m + 1])
                    a2.append(x)
                h = []
                for m in range(nHO):
                    pw = ps_w.tile([128, tile_n], F32, tag="pw")
                    for c in range(nH2):
                        nc.tensor.matmul(pw[:], we3_t[:, c, bass.ts(m, 128)], a2[c][:],
                                         start=(c == 0), stop=(c == nH2 - 1))
                    x = ap.tile([128, tile_n], BF16, tag=f"h_{m}")
                    nc.scalar.activation(x[:], pw[:], ActF.Identity,
                                         bias=bias_t[:, OB3 + m:OB3 + m + 1])
                    h.append(x)

                # K decoders, masked accumulation into po
                po = ps_o.tile([128, tile_n], F32, tag="out")
                for k in range(K):
                    d1 = []
                    for m in range(nDH):
                        pw = ps_w.tile([128, tile_n], F32, tag="pw")
                        for c in range(nHO):
                            nc.tensor.matmul(pw[:], wd1_t[:, k, c, bass.ts(m, 128)],
                                             h[c][:],
                                             start=(c == 0), stop=(c == nHO - 1))
                        x = dp.tile([128, tile_n], BF16, tag=f"d1_{m}")
                        nc.scalar.activation(
                            x[:], pw[:], ActF.Relu,
                            bias=bias_t[:, OD1 + k * nDH + m:OD1 + k * nDH + m + 1])
                        d1.append(x)
                    d2m = []
                    for m in range(nDH):
                        pw = ps_w.tile([128, tile_n], F32, tag="pw")
                        for c in range(nDH):
                            nc.tensor.matmul(pw[:], wd2_t[:, k, c, bass.ts(m, 128)],
                                             d1[c][:],
                                             start=(c == 0), stop=(c == nDH - 1))
                        x = dp.tile([128, tile_n], BF16, tag=f"d2_{m}")
                        bb = bias_t[:, OD2 + k * nDH + m:OD2 + k * nDH + m + 1]
                        # relu(psum + bias): split between DVE and ACT
                        if m % 2 == 0:
                            nc.vector.tensor_scalar(x[:], pw[:], bb, 0.0,
                                                    Alu.add, Alu.max)
                        else:
                            nc.scalar.activation(x[:], pw[:], ActF.Relu, bias=bb)
                        y = dp.tile([128, tile_n], BF16, tag=f"d2m_{m}")
                        # (cat == k) * d2 on DVE
                        nc.vector.scalar_tensor_tensor(
                            y[:], c_full[:], float(k), x[:], Alu.is_equal, Alu.mult)
                        d2m.append(y)
                    for c in range(nDH):
                        nc.tensor.matmul(po[:], wd3_t[:, k, c, :], d2m[c][:],
                                         start=(k == 0 and c == 0), stop=False,
                                         skip_group_check=True)
                # per-category output bias via the one-hot mask
                nc.tensor.matmul(po[:], bd3w_t[:], mask8[:], start=False, stop=True,
                                 skip_group_check=True)
                osb = ap.tile([128, tile_n], F32, tag="osb")
                nc.scalar.activation(osb[:], po[:], ActF.Copy)
                nc.gpsimd.dma_start(out[:, sl], osb[:])

    nc.finalize()
    return nc


def _pack_inputs(features, We1, be1, We2, be2, We3, be3,
                 Wd1, bd1, Wd2, bd2, Wd3, bd3, cat_idx):
    bf = ml_dtypes.bfloat16
    fT = np.ascontiguousarray(np.asarray(features, np.float32).T.astype(bf))
    catf = np.asarray(cat_idx).astype(np.float32).astype(bf).reshape(1, B)

    def chunkcols(b):
        b = np.asarray(b, np.float32).reshape(-1)
        return b.reshape(-1, 128).T  # [128, n]

    bias_all = np.zeros((128, NBIAS), np.float32)
    bias_all[:, OB1:OB1 + 4] = chunkcols(be1)
    bias_all[:, OB2:OB2 + 2] = chunkcols(be2)
    bias_all[:, OB3:OB3 + 2] = chunkcols(be3)
    bias_all[:, OD1:OD1 + 16] = chunkcols(bd1)
    bias_all[:, OD2:OD2 + 16] = chunkcols(bd2)
    bias_all[0:8, OIO] = np.arange(8, dtype=np.float32)

    shared = dict(
        we1=np.asarray(We1, np.float32).astype(bf),
        we2=np.asarray(We2, np.float32).astype(bf),
        we3=np.asarray(We3, np.float32).astype(bf),
        wd1=np.asarray(Wd1, np.float32).astype(bf),
        wd2=np.asarray(Wd2, np.float32).astype(bf),
        wd3=np.asarray(Wd3, np.float32).astype(bf),
        bias_all=bias_all,
        bd3w=np.asarray(bd3, np.float32).astype(bf),
        ones=np.ones((1, 128), dtype=bf),
    )
    maps = []
    for i in range(N_CORES):
        m = dict(shared)
        m["fT"] = np.ascontiguousarray(fT[:, i * B_LOC:(i + 1) * B_LOC])
        m["cat"] = np.ascontiguousarray(catf[:, i * B_LOC:(i + 1) * B_LOC])
        maps.append(m)
    return maps


_NC_CACHE = None


def _get_nc():
    global _NC_CACHE
    if _NC_CACHE is None:
        _NC_CACHE = _build_nc()
    return _NC_CACHE


def kernel(**inputs) -> np.ndarray:
    maps = _pack_inputs(**inputs)
    nc = _get_nc()
    res = bass_utils.run_bass_kernel_spmd(nc, maps, core_ids=list(range(N_CORES)))
    return np.concatenate([r["out"] for r in res.results], axis=1).T.astype(
        np.float32, copy=True)


# revision 4
# speedup vs baseline: 2.2988x; 2.2988x over previous
"""Trainium2 Bass kernel for nn_CategorySpecificInitNet (moe_routing).

kernel(**inputs) takes the FULL unsharded inputs (keys as in
reference.setup_inputs()) and returns the FULL [B, 128] float32 output.

Strategy — expert-parallel, per the spec sharding hint's dispatch-by-category
alternative:
  - the host sharding layer dispatches rows to cores by category (the
    "all-to-all dispatch by category" of expert-parallel, realized where
    all sharding happens in this harness): rows are stably sorted by
    cat_idx and core k receives category k's rows, zero-padded to a
    static per-core capacity (max category count rounded up to the
    512-row tile size);
  - every core runs the shared encoder plus exactly ONE decoder (its
    category's), so no routing, masking, or gather happens per row —
    the decoder FLOPs drop 8x vs computing all decoders densely;
  - all compute is feature-major [features(partitions), rows(free)], so
    no transposes are ever needed on device (the host passes features
    pre-transposed); outputs come back [128, cap] and the host
    inverse-permutes rows during unsharding.
  - per-core row tiles of 512; the decoder stages are software-pipelined
    one tile behind the encoder so the PE never waits on ACT/DVE
    relu latency.

Matmuls run in float32r (fp32 storage, full PE rate at N=512, ~tf32-grade
multiply precision on HW — measured ~3e-4 max rel error vs the fp32
reference, 17x better than bf16 at the same PE throughput).
"""
import sys

for _p in ("/opt/trn_rl_repo",):
    if _p not in sys.path:
        sys.path.append(_p)

import numpy as np

import concourse.bass as bass
import concourse.bacc as bacc
import concourse.mybir as mybir
import concourse.tile as tile
from concourse import bass_utils

FR = mybir.dt.float32r
F32 = mybir.dt.float32
Alu = mybir.AluOpType
ActF = mybir.ActivationFunctionType

B, C, H1, H2, HO = 32768, 768, 512, 256, 256
DH, LAT, K = 256, 128, 8
N_CORES = 8
TILE = 512
# bias_all columns: be1[4] be2[2] be3[2] bd1[2] bd2[2] bd3[1]
OB1, OB2, OB3, OD1, OD2, OD3 = 0, 4, 6, 8, 10, 12
NBIAS = 13


def _build_nc(cap, tile_n=TILE, ps_w_bufs=6, dp_bufs=3):
    assert cap % tile_n == 0
    nt = cap // tile_n
    nc = bacc.Bacc(name="catnet_ep")

    fT = nc.dram_tensor("fT", (C, cap), FR, kind="ExternalInput")
    we1 = nc.dram_tensor("we1", (C, H1), FR, kind="ExternalInput")
    we2 = nc.dram_tensor("we2", (H1, H2), FR, kind="ExternalInput")
    we3 = nc.dram_tensor("we3", (H2, HO), FR, kind="ExternalInput")
    wd1 = nc.dram_tensor("wd1", (HO, DH), FR, kind="ExternalInput")
    wd2 = nc.dram_tensor("wd2", (DH, DH), FR, kind="ExternalInput")
    wd3 = nc.dram_tensor("wd3", (DH, LAT), FR, kind="ExternalInput")
    bias_all = nc.dram_tensor("bias_all", (128, NBIAS), F32, kind="ExternalInput")
    out = nc.dram_tensor("out", (LAT, cap), F32, kind="ExternalOutput")

    nC, nH1, nH2, nHO, nDH = C // 128, H1 // 128, H2 // 128, HO // 128, DH // 128

    with tile.TileContext(nc) as tc:
        with (
            tc.tile_pool(name="wp", bufs=1) as wp,
            tc.tile_pool(name="fp", bufs=2) as fp,
            tc.tile_pool(name="ap", bufs=2) as ap,
            tc.tile_pool(name="dp", bufs=dp_bufs) as dp,
            tc.tile_pool(name="ps_w", bufs=ps_w_bufs, space="PSUM") as ps_w,
            tc.tile_pool(name="ps_o", bufs=2, space="PSUM") as ps_o,
        ):
            bias_t = wp.tile([128, NBIAS], F32, tag="bias")
            nc.sync.dma_start(bias_t[:], bias_all[:])

            # we1 split per chunk: the first L1 matmuls start as soon as
            # their contraction chunk lands instead of after the whole tensor
            we1_t = wp.tile([128, nC, H1], FR, tag="we1")
            for c in range(nC):
                nc.gpsimd.dma_start(we1_t[:, c, :], we1[c * 128:(c + 1) * 128, :])
            we2_t = wp.tile([128, nH1, H2], FR, tag="we2")
            nc.gpsimd.dma_start(we2_t[:], we2.rearrange("(c p) h -> p c h", p=128))
            we3_t = wp.tile([128, nH2, HO], FR, tag="we3")
            nc.gpsimd.dma_start(we3_t[:], we3.rearrange("(c p) h -> p c h", p=128))
            wd1_t = wp.tile([128, nHO, DH], FR, tag="wd1")
            nc.gpsimd.dma_start(wd1_t[:], wd1.rearrange("(c p) d -> p c d", p=128))
            wd2_t = wp.tile([128, nDH, DH], FR, tag="wd2")
            nc.gpsimd.dma_start(wd2_t[:], wd2.rearrange("(c p) d -> p c d", p=128))
            wd3_t = wp.tile([128, nDH, LAT], FR, tag="wd3")
            nc.gpsimd.dma_start(wd3_t[:], wd3.rearrange("(c p) d -> p c d", p=128))

            def emit_enc(t):
                sl = bass.ts(t, tile_n)
                ftb = fp.tile([128, nC, tile_n], FR, tag="ft")
                if t == 0:
                    for c in range(nC):
                        nc.sync.dma_start(ftb[:, c, :],
                                          fT[c * 128:(c + 1) * 128, sl])
                else:
                    nc.sync.dma_start(
                        ftb[:], fT.rearrange("(c p) b -> p c b", p=128)[:, :, sl])
                a1 = []
                for m in range(nH1):
                    pw = ps_w.tile([128, tile_n], F32, tag="pw")
                    for c in range(nC):
                        nc.tensor.matmul(pw[:], we1_t[:, c, bass.ts(m, 128)],
                                         ftb[:, c, :],
                                         start=(c == 0), stop=(c == nC - 1))
                    x = ap.tile([128, tile_n], FR, tag=f"a1_{m}")
                    nc.scalar.activation(x[:], pw[:], ActF.Relu,
                                         bias=bias_t[:, OB1 + m:OB1 + m + 1])
                    a1.append(x)
                a2 = []
                for m in range(nH2):
                    pw = ps_w.tile([128, tile_n], F32, tag="pw")
                    for c in range(nH1):
                        nc.tensor.matmul(pw[:], we2_t[:, c, bass.ts(m, 128)], a1[c][:],
                                         start=(c == 0), stop=(c == nH1 - 1))
                    x = ap.tile([128, tile_n], FR, tag=f"a2_{m}")
                    nc.scalar.activation(x[:], pw[:], ActF.Relu,
                                         bias=bias_t[:, OB2 + m:OB2 + m + 1])
                    a2.append(x)
                h = []
                for m in range(nHO):
                    pw = ps_w.tile([128, tile_n], F32, tag="pw")
                    for c in range(nH2):
                        nc.tensor.matmul(pw[:], we3_t[:, c, bass.ts(m, 128)], a2[c][:],
                                         start=(c == 0), stop=(c == nH2 - 1))
                    x = ap.tile([128, tile_n], FR, tag=f"h_{m}")
                    nc.scalar.activation(x[:], pw[:], ActF.Identity,
                                         bias=bias_t[:, OB3 + m:OB3 + m + 1])
                    h.append(x)
                return h

            def emit_d1(h):
                d1 = []
                for m in range(nDH):
                    pw = ps_w.tile([128, tile_n], F32, tag="pw")
                    for c in range(nHO):
                        nc.tensor.matmul(pw[:], wd1_t[:, c, bass.ts(m, 128)], h[c][:],
                                         start=(c == 0), stop=(c == nHO - 1))
                    x = dp.tile([128, tile_n], FR, tag=f"d1_{m}")
                    nc.scalar.activation(x[:], pw[:], ActF.Relu,
                                         bias=bias_t[:, OD1 + m:OD1 + m + 1])
                    d1.append(x)
                return d1

            def emit_d2_d3_store(t, d1):
                d2 = []
                for m in range(nDH):
                    pw = ps_w.tile([128, tile_n], F32, tag="pw")
                    for c in range(nDH):
                        nc.tensor.matmul(pw[:], wd2_t[:, c, bass.ts(m, 128)], d1[c][:],
                                         start=(c == 0), stop=(c == nDH - 1))
                    x = dp.tile([128, tile_n], FR, tag=f"d2_{m}")
                    bb = bias_t[:, OD2 + m:OD2 + m + 1]
                    if m % 2 == 0:
                        nc.vector.tensor_scalar(x[:], pw[:], bb, 0.0, Alu.add, Alu.max)
                    else:
                        nc.scalar.activation(x[:], pw[:], ActF.Relu, bias=bb)
                    d2.append(x)
                po = ps_o.tile([128, tile_n], F32, tag="out")
                for c in range(nDH):
                    nc.tensor.matmul(po[:], wd3_t[:, c, :], d2[c][:],
                                     start=(c == 0), stop=(c == nDH - 1))
                osb = ap.tile([128, tile_n], F32, tag="osb")
                nc.scalar.activation(osb[:], po[:], ActF.Identity,
                                     bias=bias_t[:, OD3:OD3 + 1])
                nc.gpsimd.dma_start(out[:, bass.ts(t, tile_n)], osb[:])

            # decoder runs one tile behind the encoder: PE order per step is
            # [enc t][d2/d3 t-1][d1 t], hiding ACT latency behind matmuls
            pend = None
            for t in range(nt):
                h = emit_enc(t)
                if pend is not None:
                    emit_d2_d3_store(pend[0], pend[1])
                d1 = emit_d1(h)
                pend = (t, d1)
            emit_d2_d3_store(pend[0], pend[1])

    nc.finalize()
    return nc


def _pack_inputs(features, We1, be1, We2, be2, We3, be3,
                 Wd1, bd1, Wd2, bd2, Wd3, bd3, cat_idx, cap):
    """Dispatch rows to cores by category (expert-parallel sharding)."""
    features = np.asarray(features, np.float32)
    cat = np.asarray(cat_idx).astype(np.int64)
    order = np.argsort(cat, kind="stable")
    counts = np.bincount(cat, minlength=N_CORES)
    starts = np.zeros(N_CORES + 1, np.int64)
    np.cumsum(counts, out=starts[1:])

    def chunkcols(b):
        b = np.asarray(b, np.float32).reshape(-1)
        return b.reshape(-1, 128).T

    enc = dict(
        we1=np.asarray(We1, np.float32), we2=np.asarray(We2, np.float32),
        we3=np.asarray(We3, np.float32),
    )
    maps, rows_per_core = [], []
    for k in range(N_CORES):
        rows = order[starts[k]:starts[k + 1]]
        rows_per_core.append(rows)
        f = np.zeros((cap, C), np.float32)
        f[:len(rows)] = features[rows]
        bias_all = np.zeros((128, NBIAS), np.float32)
        bias_all[:, OB1:OB1 + 4] = chunkcols(be1)
        bias_all[:, OB2:OB2 + 2] = chunkcols(be2)
        bias_all[:, OB3:OB3 + 2] = chunkcols(be3)
        bias_all[:, OD1:OD1 + 2] = chunkcols(np.asarray(bd1, np.float32)[k])
        bias_all[:, OD2:OD2 + 2] = chunkcols(np.asarray(bd2, np.float32)[k])
        bias_all[:, OD3:OD3 + 1] = chunkcols(np.asarray(bd3, np.float32)[k])
        m = dict(enc)
        m["fT"] = np.ascontiguousarray(f.T)
        m["wd1"] = np.asarray(Wd1, np.float32)[k]
        m["wd2"] = np.asarray(Wd2, np.float32)[k]
        m["wd3"] = np.asarray(Wd3, np.float32)[k]
        m["bias_all"] = bias_all
        maps.append(m)
    return maps, rows_per_core


_NC_CACHE = {}


def _get_nc(cap=4608):
    if cap not in _NC_CACHE:
        _NC_CACHE[cap] = _build_nc(cap)
    return _NC_CACHE[cap]


def kernel(**inputs) -> np.ndarray:
    cat = np.asarray(inputs["cat_idx"]).astype(np.int64)
    counts = np.bincount(cat, minlength=N_CORES)
    cap = max(TILE, int(-(-counts.max() // TILE) * TILE))
    maps, rows_per_core = _pack_inputs(**inputs, cap=cap)
    nc = _get_nc(cap)
    res = bass_utils.run_bass_kernel_spmd(nc, maps, core_ids=list(range(N_CORES)))
    latent = np.zeros((B, LAT), np.float32)
    for k, r in enumerate(res.results):
        rows = rows_per_core[k]
        latent[rows] = r["out"][:, :len(rows)].T
    return latent


# revision 5
# speedup vs baseline: 2.4136x; 1.0499x over previous
"""Trainium2 Bass kernel for nn_CategorySpecificInitNet (moe_routing).

kernel(**inputs) takes the FULL unsharded inputs (keys as in
reference.setup_inputs()) and returns the FULL [B, 128] float32 output.

Strategy — expert-parallel, per the spec sharding hint's dispatch-by-category
alternative:
  - the host sharding layer dispatches rows to cores by category (the
    "all-to-all dispatch by category" of expert-parallel, realized where
    all sharding happens in this harness): rows are stably sorted by
    cat_idx and core k receives category k's rows, zero-padded to a
    static per-core capacity (max category count rounded up to the
    512-row tile size);
  - every core runs the shared encoder plus exactly ONE decoder (its
    category's), so no routing, masking, or gather happens per row —
    the decoder FLOPs drop 8x vs computing all decoders densely;
  - all compute is feature-major [features(partitions), rows(free)], so
    no transposes are ever needed on device (the host passes features
    pre-transposed); outputs come back [128, cap] and the host
    inverse-permutes rows during unsharding.
  - per-core row tiles of 512; the decoder stages are software-pipelined
    one tile behind the encoder so the PE never waits on ACT/DVE
    relu latency.

Matmuls run in float32r (fp32 storage, full PE rate at N=512, ~tf32-grade
multiply precision on HW — measured ~3e-4 max rel error vs the fp32
reference, 17x better than bf16 at the same PE throughput).
"""
import sys

for _p in ("/opt/trn_rl_repo",):
    if _p not in sys.path:
        sys.path.append(_p)

import numpy as np

import concourse.bass as bass
import concourse.bacc as bacc
import concourse.mybir as mybir
import concourse.tile as tile
from concourse import bass_utils

FR = mybir.dt.float32r
F32 = mybir.dt.float32
Alu = mybir.AluOpType
ActF = mybir.ActivationFunctionType

B, C, H1, H2, HO = 32768, 768, 512, 256, 256
DH, LAT, K = 256, 128, 8
N_CORES = 8
TILE = 512
# bias_all columns: be1[4] be2[2] be3[2] bd1[2] bd2[2] bd3[1]
OB1, OB2, OB3, OD1, OD2, OD3 = 0, 4, 6, 8, 10, 12
NBIAS = 13


def _build_nc(cap, tile_n=512, ps_w_bufs=6, dp_bufs=3):
    assert cap % 256 == 0
    tiles = [tile_n] * (cap // tile_n)
    if cap % tile_n:
        tiles.append(cap % tile_n)
    offs = [sum(tiles[:i]) for i in range(len(tiles))]
    nt = len(tiles)
    nc = bacc.Bacc(name="catnet_ep")

    fT = nc.dram_tensor("fT", (C, cap), FR, kind="ExternalInput")
    we1 = nc.dram_tensor("we1", (C, H1), FR, kind="ExternalInput")
    we2 = nc.dram_tensor("we2", (H1, H2), FR, kind="ExternalInput")
    we3 = nc.dram_tensor("we3", (H2, HO), FR, kind="ExternalInput")
    wd1 = nc.dram_tensor("wd1", (HO, DH), FR, kind="ExternalInput")
    wd2 = nc.dram_tensor("wd2", (DH, DH), FR, kind="ExternalInput")
    wd3 = nc.dram_tensor("wd3", (DH, LAT), FR, kind="ExternalInput")
    bias_all = nc.dram_tensor("bias_all", (128, NBIAS), F32, kind="ExternalInput")
    out = nc.dram_tensor("out", (LAT, cap), F32, kind="ExternalOutput")

    nC, nH1, nH2, nHO, nDH = C // 128, H1 // 128, H2 // 128, HO // 128, DH // 128

    with tile.TileContext(nc) as tc:
        with (
            tc.tile_pool(name="wp", bufs=1) as wp,
            tc.tile_pool(name="fp", bufs=2) as fp,
            tc.tile_pool(name="ap", bufs=2) as ap,
            tc.tile_pool(name="dp", bufs=dp_bufs) as dp,
            tc.tile_pool(name="ps_w", bufs=ps_w_bufs, space="PSUM") as ps_w,
            tc.tile_pool(name="ps_o", bufs=2, space="PSUM") as ps_o,
        ):
            bias_t = wp.tile([128, NBIAS], F32, tag="bias")
            nc.sync.dma_start(bias_t[:], bias_all[:])

            # we1 split per chunk: the first L1 matmuls start as soon as
            # their contraction chunk lands instead of after the whole tensor
            we1_t = wp.tile([128, nC, H1], FR, tag="we1")
            for c in range(nC):
                nc.gpsimd.dma_start(we1_t[:, c, :], we1[c * 128:(c + 1) * 128, :])
            we2_t = wp.tile([128, nH1, H2], FR, tag="we2")
            nc.gpsimd.dma_start(we2_t[:], we2.rearrange("(c p) h -> p c h", p=128))
            we3_t = wp.tile([128, nH2, HO], FR, tag="we3")
            nc.gpsimd.dma_start(we3_t[:], we3.rearrange("(c p) h -> p c h", p=128))
            wd1_t = wp.tile([128, nHO, DH], FR, tag="wd1")
            nc.gpsimd.dma_start(wd1_t[:], wd1.rearrange("(c p) d -> p c d", p=128))
            wd2_t = wp.tile([128, nDH, DH], FR, tag="wd2")
            nc.gpsimd.dma_start(wd2_t[:], wd2.rearrange("(c p) d -> p c d", p=128))
            wd3_t = wp.tile([128, nDH, LAT], FR, tag="wd3")
            nc.gpsimd.dma_start(wd3_t[:], wd3.rearrange("(c p) d -> p c d", p=128))

            def emit_enc(t):
                tn = tiles[t]
                sl = slice(offs[t], offs[t] + tn)
                ftb_fl = fp.tile([128, nC, tile_n], FR, tag="ft")
                ftb = ftb_fl[:, :, :tn]
                if t == 0:
                    for c in range(nC):
                        nc.sync.dma_start(ftb[:, c, :],
                                          fT[c * 128:(c + 1) * 128, sl])
                else:
                    nc.sync.dma_start(
                        ftb[:], fT.rearrange("(c p) b -> p c b", p=128)[:, :, sl])
                pwsl = slice(0, tn)
                a1 = []
                for m in range(nH1):
                    pw = ps_w.tile([128, tile_n], F32, tag="pw")[:, :tn]
                    for c in range(nC):
                        nc.tensor.matmul(pw[:], we1_t[:, c, bass.ts(m, 128)],
                                         ftb[:, c, :],
                                         start=(c == 0), stop=(c == nC - 1))
                    x = ap.tile([128, tile_n], FR, tag=f"a1_{m}")[:, :tn]
                    nc.scalar.activation(x[:], pw[:], ActF.Relu,
                                         bias=bias_t[:, OB1 + m:OB1 + m + 1])
                    a1.append(x)
                a2 = []
                for m in range(nH2):
                    pw = ps_w.tile([128, tile_n], F32, tag="pw")[:, :tn]
                    for c in range(nH1):
                        nc.tensor.matmul(pw[:], we2_t[:, c, bass.ts(m, 128)], a1[c][:],
                                         start=(c == 0), stop=(c == nH1 - 1))
                    x = ap.tile([128, tile_n], FR, tag=f"a2_{m}")[:, :tn]
                    nc.scalar.activation(x[:], pw[:], ActF.Relu,
                                         bias=bias_t[:, OB2 + m:OB2 + m + 1])
                    a2.append(x)
                h = []
                for m in range(nHO):
                    pw = ps_w.tile([128, tile_n], F32, tag="pw")[:, :tn]
                    for c in range(nH2):
                        nc.tensor.matmul(pw[:], we3_t[:, c, bass.ts(m, 128)], a2[c][:],
                                         start=(c == 0), stop=(c == nH2 - 1))
                    x = ap.tile([128, tile_n], FR, tag=f"h_{m}")[:, :tn]
                    nc.scalar.activation(x[:], pw[:], ActF.Identity,
                                         bias=bias_t[:, OB3 + m:OB3 + m + 1])
                    h.append(x)
                return h

            def emit_d1(t, h):
                tn = tiles[t]
                d1 = []
                for m in range(nDH):
                    pw = ps_w.tile([128, tile_n], F32, tag="pw")[:, :tn]
                    for c in range(nHO):
                        nc.tensor.matmul(pw[:], wd1_t[:, c, bass.ts(m, 128)], h[c][:],
                                         start=(c == 0), stop=(c == nHO - 1))
                    x = dp.tile([128, tile_n], FR, tag=f"d1_{m}")[:, :tn]
                    nc.scalar.activation(x[:], pw[:], ActF.Relu,
                                         bias=bias_t[:, OD1 + m:OD1 + m + 1])
                    d1.append(x)
                return d1

            def emit_d2_d3_store(t, d1):
                tn = tiles[t]
                d2 = []
                for m in range(nDH):
                    pw = ps_w.tile([128, tile_n], F32, tag="pw")[:, :tn]
                    for c in range(nDH):
                        nc.tensor.matmul(pw[:], wd2_t[:, c, bass.ts(m, 128)], d1[c][:],
                                         start=(c == 0), stop=(c == nDH - 1))
                    x = dp.tile([128, tile_n], FR, tag=f"d2_{m}")[:, :tn]
                    bb = bias_t[:, OD2 + m:OD2 + m + 1]
                    if m % 2 == 0:
                        nc.vector.tensor_scalar(x[:], pw[:], bb, 0.0, Alu.add, Alu.max)
                    else:
                        nc.scalar.activation(x[:], pw[:], ActF.Relu, bias=bb)
                    d2.append(x)
                po = ps_o.tile([128, tile_n], F32, tag="out")[:, :tn]
                for c in range(nDH):
                    nc.tensor.matmul(po[:], wd3_t[:, c, :], d2[c][:],
                                     start=(c == 0), stop=(c == nDH - 1))
                osb = ap.tile([128, tile_n], F32, tag="osb")[:, :tn]
                nc.scalar.activation(osb[:], po[:], ActF.Identity,
                                     bias=bias_t[:, OD3:OD3 + 1])
                nc.gpsimd.dma_start(out[:, offs[t]:offs[t] + tn], osb[:])

            # decoder runs one tile behind the encoder: PE order per step is
            # [enc t][d2/d3 t-1][d1 t], hiding ACT latency behind matmuls
            pend = None
            for t in range(nt):
                h = emit_enc(t)
                if pend is not None:
                    emit_d2_d3_store(pend[0], pend[1])
                d1 = emit_d1(t, h)
                pend = (t, d1)
            emit_d2_d3_store(pend[0], pend[1])

    nc.finalize()
    return nc


def _pack_inputs(features, We1, be1, We2, be2, We3, be3,
                 Wd1, bd1, Wd2, bd2, Wd3, bd3, cat_idx, cap):
    """Dispatch rows to cores by category (expert-parallel sharding)."""
    features = np.asarray(features, np.float32)
    cat = np.asarray(cat_idx).astype(np.int64)
    order = np.argsort(cat, kind="stable")
    counts = np.bincount(cat, minlength=N_CORES)
    starts = np.zeros(N_CORES + 1, np.int64)
    np.cumsum(counts, out=starts[1:])

    def chunkcols(b):
        b = np.asarray(b, np.float32).reshape(-1)
        return b.reshape(-1, 128).T

    enc = dict(
        we1=np.asarray(We1, np.float32), we2=np.asarray(We2, np.float32),
        we3=np.asarray(We3, np.float32),
    )
    maps, rows_per_core = [], []
    for k in range(N_CORES):
        rows = order[starts[k]:starts[k + 1]]
        rows_per_core.append(rows)
        f = np.zeros((cap, C), np.float32)
        f[:len(rows)] = features[rows]
        bias_all = np.zeros((128, NBIAS), np.float32)
        bias_all[:, OB1:OB1 + 4] = chunkcols(be1)
        bias_all[:, OB2:OB2 + 2] = chunkcols(be2)
        bias_all[:, OB3:OB3 + 2] = chunkcols(be3)
        bias_all[:, OD1:OD1 + 2] = chunkcols(np.asarray(bd1, np.float32)[k])
        bias_all[:, OD2:OD2 + 2] = chunkcols(np.asarray(bd2, np.float32)[k])
        bias_all[:, OD3:OD3 + 1] = chunkcols(np.asarray(bd3, np.float32)[k])
        m = dict(enc)
        m["fT"] = np.ascontiguousarray(f.T)
        m["wd1"] = np.asarray(Wd1, np.float32)[k]
        m["wd2"] = np.asarray(Wd2, np.float32)[k]
        m["wd3"] = np.asarray(Wd3, np.float32)[k]
        m["bias_all"] = bias_all
        maps.append(m)
    return maps, rows_per_core


_NC_CACHE = {}


def _get_nc(cap=4352):
    if cap not in _NC_CACHE:
        _NC_CACHE[cap] = _build_nc(cap)
    return _NC_CACHE[cap]


def kernel(**inputs) -> np.ndarray:
    cat = np.asarray(inputs["cat_idx"]).astype(np.int64)
    counts = np.bincount(cat, minlength=N_CORES)
    cap = max(256, int(-(-counts.max() // 256) * 256))
    maps, rows_per_core = _pack_inputs(**inputs, cap=cap)
    nc = _get_nc(cap)
    res = bass_utils.run_bass_kernel_spmd(nc, maps, core_ids=list(range(N_CORES)))
    latent = np.zeros((B, LAT), np.float32)
    for k, r in enumerate(res.results):
        rows = rows_per_core[k]
        latent[rows] = r["out"][:, :len(rows)].T
    return latent


# revision 6
# speedup vs baseline: 2.5471x; 1.0553x over previous
"""Trainium2 Bass kernel for nn_CategorySpecificInitNet (moe_routing).

kernel(**inputs) takes the FULL unsharded inputs (keys as in
reference.setup_inputs()) and returns the FULL [B, 128] float32 output.

Strategy — expert-parallel, per the spec sharding hint's dispatch-by-category
alternative:
  - the host sharding layer dispatches rows to cores by category (the
    "all-to-all dispatch by category" of expert-parallel, realized where
    all sharding happens in this harness): rows are stably sorted by
    cat_idx and core k receives category k's rows, zero-padded to a
    static per-core capacity (max category count rounded up to the
    512-row tile size);
  - every core runs the shared encoder plus exactly ONE decoder (its
    category's), so no routing, masking, or gather happens per row —
    the decoder FLOPs drop 8x vs computing all decoders densely;
  - all compute is feature-major [features(partitions), rows(free)], so
    no transposes are ever needed on device (the host passes features
    pre-transposed); outputs come back [128, cap] and the host
    inverse-permutes rows during unsharding.
  - per-core row tiles of 512; the decoder stages are software-pipelined
    one tile behind the encoder so the PE never waits on ACT/DVE
    relu latency.

Matmuls run in float32r (fp32 storage, full PE rate at N=512, ~tf32-grade
multiply precision on HW — measured ~3e-4 max rel error vs the fp32
reference, 17x better than bf16 at the same PE throughput).
"""
import sys

for _p in ("/opt/trn_rl_repo",):
    if _p not in sys.path:
        sys.path.append(_p)

import numpy as np

import concourse.bass as bass
import concourse.bacc as bacc
import concourse.mybir as mybir
import concourse.tile as tile
from concourse import bass_utils

FR = mybir.dt.float32r
F32 = mybir.dt.float32
Alu = mybir.AluOpType
ActF = mybir.ActivationFunctionType

B, C, H1, H2, HO = 32768, 768, 512, 256, 256
DH, LAT, K = 256, 128, 8
N_CORES = 8
TILE = 512
# bias_all columns: be1[4] be2[2] be3[2] bd1[2] bd2[2] bd3[1]
OB1, OB2, OB3, OD1, OD2, OD3 = 0, 4, 6, 8, 10, 12
NBIAS = 13


def _build_nc(cap, tile_n=512, ps_w_bufs=6, dp_bufs=3):
    assert cap % 256 == 0
    tiles = [tile_n] * (cap // tile_n)
    if cap % tile_n:
        tiles.append(cap % tile_n)
    offs = [sum(tiles[:i]) for i in range(len(tiles))]
    nt = len(tiles)
    nc = bacc.Bacc(name="catnet_ep")

    fT = nc.dram_tensor("fT", (C, cap), FR, kind="ExternalInput")
    we1 = nc.dram_tensor("we1", (C, H1), FR, kind="ExternalInput")
    we2 = nc.dram_tensor("we2", (H1, H2), FR, kind="ExternalInput")
    we3 = nc.dram_tensor("we3", (H2, HO), FR, kind="ExternalInput")
    wd1 = nc.dram_tensor("wd1", (HO, DH), FR, kind="ExternalInput")
    wd2 = nc.dram_tensor("wd2", (DH, DH), FR, kind="ExternalInput")
    wd3 = nc.dram_tensor("wd3", (DH, LAT), FR, kind="ExternalInput")
    bias_all = nc.dram_tensor("bias_all", (128, NBIAS), F32, kind="ExternalInput")
    out = nc.dram_tensor("out", (LAT, cap), F32, kind="ExternalOutput")

    nC, nH1, nH2, nHO, nDH = C // 128, H1 // 128, H2 // 128, HO // 128, DH // 128

    with tile.TileContext(nc) as tc:
        with (
            tc.tile_pool(name="wp", bufs=1) as wp,
            tc.tile_pool(name="fp", bufs=2) as fp,
            tc.tile_pool(name="ap", bufs=2) as ap,
            tc.tile_pool(name="dp", bufs=dp_bufs) as dp,
            tc.tile_pool(name="ps_w", bufs=ps_w_bufs, space="PSUM") as ps_w,
            tc.tile_pool(name="ps_o", bufs=2, space="PSUM") as ps_o,
        ):
            bias_t = wp.tile([128, NBIAS], F32, tag="bias")
            nc.gpsimd.dma_start(bias_t[:], bias_all[:])

            # we1 split per chunk: the first L1 matmuls start as soon as
            # their contraction chunk lands instead of after the whole tensor
            we1_t = wp.tile([128, nC, H1], FR, tag="we1")
            for c in range(nC):
                nc.gpsimd.dma_start(we1_t[:, c, :], we1[c * 128:(c + 1) * 128, :])
            we2_t = wp.tile([128, nH1, H2], FR, tag="we2")
            nc.gpsimd.dma_start(we2_t[:], we2.rearrange("(c p) h -> p c h", p=128))
            we3_t = wp.tile([128, nH2, HO], FR, tag="we3")
            nc.gpsimd.dma_start(we3_t[:], we3.rearrange("(c p) h -> p c h", p=128))
            wd1_t = wp.tile([128, nHO, DH], FR, tag="wd1")
            nc.gpsimd.dma_start(wd1_t[:], wd1.rearrange("(c p) d -> p c d", p=128))
            wd2_t = wp.tile([128, nDH, DH], FR, tag="wd2")
            nc.gpsimd.dma_start(wd2_t[:], wd2.rearrange("(c p) d -> p c d", p=128))
            wd3_t = wp.tile([128, nDH, LAT], FR, tag="wd3")
            nc.gpsimd.dma_start(wd3_t[:], wd3.rearrange("(c p) d -> p c d", p=128))

            def emit_enc(t):
                tn = tiles[t]
                sl = slice(offs[t], offs[t] + tn)
                ftb_fl = fp.tile([128, nC, tile_n], FR, tag="ft")
                ftb = ftb_fl[:, :, :tn]
                if t == 0:
                    for c in range(nC):
                        nc.sync.dma_start(ftb[:, c, :],
                                          fT[c * 128:(c + 1) * 128, sl])
                else:
                    # alternate queues so consecutive feature tiles stream in
                    # parallel instead of serializing on one HWDGE queue
                    eng = nc.sync if t % 2 == 0 else nc.gpsimd
                    eng.dma_start(
                        ftb[:], fT.rearrange("(c p) b -> p c b", p=128)[:, :, sl])
                pwsl = slice(0, tn)
                a1 = []
                for m in range(nH1):
                    pw = ps_w.tile([128, tile_n], F32, tag="pw")[:, :tn]
                    for c in range(nC):
                        nc.tensor.matmul(pw[:], we1_t[:, c, bass.ts(m, 128)],
                                         ftb[:, c, :],
                                         start=(c == 0), stop=(c == nC - 1))
                    x = ap.tile([128, tile_n], FR, tag=f"a1_{m}")[:, :tn]
                    nc.scalar.activation(x[:], pw[:], ActF.Relu,
                                         bias=bias_t[:, OB1 + m:OB1 + m + 1])
                    a1.append(x)
                a2 = []
                for m in range(nH2):
                    pw = ps_w.tile([128, tile_n], F32, tag="pw")[:, :tn]
                    for c in range(nH1):
                        nc.tensor.matmul(pw[:], we2_t[:, c, bass.ts(m, 128)], a1[c][:],
                                         start=(c == 0), stop=(c == nH1 - 1))
                    x = ap.tile([128, tile_n], FR, tag=f"a2_{m}")[:, :tn]
                    nc.scalar.activation(x[:], pw[:], ActF.Relu,
                                         bias=bias_t[:, OB2 + m:OB2 + m + 1])
                    a2.append(x)
                h = []
                for m in range(nHO):
                    pw = ps_w.tile([128, tile_n], F32, tag="pw")[:, :tn]
                    for c in range(nH2):
                        nc.tensor.matmul(pw[:], we3_t[:, c, bass.ts(m, 128)], a2[c][:],
                                         start=(c == 0), stop=(c == nH2 - 1))
                    x = ap.tile([128, tile_n], FR, tag=f"h_{m}")[:, :tn]
                    nc.scalar.activation(x[:], pw[:], ActF.Identity,
                                         bias=bias_t[:, OB3 + m:OB3 + m + 1])
                    h.append(x)
                return h

            def emit_d1(t, h):
                tn = tiles[t]
                d1 = []
                for m in range(nDH):
                    pw = ps_w.tile([128, tile_n], F32, tag="pw")[:, :tn]
                    for c in range(nHO):
                        nc.tensor.matmul(pw[:], wd1_t[:, c, bass.ts(m, 128)], h[c][:],
                                         start=(c == 0), stop=(c == nHO - 1))
                    x = dp.tile([128, tile_n], FR, tag=f"d1_{m}")[:, :tn]
                    nc.scalar.activation(x[:], pw[:], ActF.Relu,
                                         bias=bias_t[:, OD1 + m:OD1 + m + 1])
                    d1.append(x)
                return d1

            def emit_d2_d3_store(t, d1):
                tn = tiles[t]
                d2 = []
                for m in range(nDH):
                    pw = ps_w.tile([128, tile_n], F32, tag="pw")[:, :tn]
                    for c in range(nDH):
                        nc.tensor.matmul(pw[:], wd2_t[:, c, bass.ts(m, 128)], d1[c][:],
                                         start=(c == 0), stop=(c == nDH - 1))
                    x = dp.tile([128, tile_n], FR, tag=f"d2_{m}")[:, :tn]
                    bb = bias_t[:, OD2 + m:OD2 + m + 1]
                    if m % 2 == 0:
                        nc.vector.tensor_scalar(x[:], pw[:], bb, 0.0, Alu.add, Alu.max)
                    else:
                        nc.scalar.activation(x[:], pw[:], ActF.Relu, bias=bb)
                    d2.append(x)
                po = ps_o.tile([128, tile_n], F32, tag="out")[:, :tn]
                for c in range(nDH):
                    nc.tensor.matmul(po[:], wd3_t[:, c, :], d2[c][:],
                                     start=(c == 0), stop=(c == nDH - 1))
                osb = ap.tile([128, tile_n], F32, tag="osb")[:, :tn]
                nc.scalar.activation(osb[:], po[:], ActF.Identity,
                                     bias=bias_t[:, OD3:OD3 + 1])
                nc.gpsimd.dma_start(out[:, offs[t]:offs[t] + tn], osb[:])

            # decoder runs one tile behind the encoder: PE order per step is
            # [enc t][d2/d3 t-1][d1 t], hiding ACT latency behind matmuls
            pend = None
            for t in range(nt):
                h = emit_enc(t)
                if pend is not None:
                    emit_d2_d3_store(pend[0], pend[1])
                d1 = emit_d1(t, h)
                pend = (t, d1)
            emit_d2_d3_store(pend[0], pend[1])

    nc.finalize()
    return nc


def _pack_inputs(features, We1, be1, We2, be2, We3, be3,
                 Wd1, bd1, Wd2, bd2, Wd3, bd3, cat_idx, cap):
    """Dispatch rows to cores by category (expert-parallel sharding)."""
    features = np.asarray(features, np.float32)
    cat = np.asarray(cat_idx).astype(np.int64)
    order = np.argsort(cat, kind="stable")
    counts = np.bincount(cat, minlength=N_CORES)
    starts = np.zeros(N_CORES + 1, np.int64)
    np.cumsum(counts, out=starts[1:])

    def chunkcols(b):
        b = np.asarray(b, np.float32).reshape(-1)
        return b.reshape(-1, 128).T

    enc = dict(
        we1=np.asarray(We1, np.float32), we2=np.asarray(We2, np.float32),
        we3=np.asarray(We3, np.float32),
    )
    maps, rows_per_core = [], []
    for k in range(N_CORES):
        rows = order[starts[k]:starts[k + 1]]
        rows_per_core.append(rows)
        f = np.zeros((cap, C), np.float32)
        f[:len(rows)] = features[rows]
        bias_all = np.zeros((128, NBIAS), np.float32)
        bias_all[:, OB1:OB1 + 4] = chunkcols(be1)
        bias_all[:, OB2:OB2 + 2] = chunkcols(be2)
        bias_all[:, OB3:OB3 + 2] = chunkcols(be3)
        bias_all[:, OD1:OD1 + 2] = chunkcols(np.asarray(bd1, np.float32)[k])
        bias_all[:, OD2:OD2 + 2] = chunkcols(np.asarray(bd2, np.float32)[k])
        bias_all[:, OD3:OD3 + 1] = chunkcols(np.asarray(bd3, np.float32)[k])
        m = dict(enc)
        m["fT"] = np.ascontiguousarray(f.T)
        m["wd1"] = np.asarray(Wd1, np.float32)[k]
        m["wd2"] = np.asarray(Wd2, np.float32)[k]
        m["wd3"] = np.asarray(Wd3, np.float32)[k]
        m["bias_all"] = bias_all
        maps.append(m)
    return maps, rows_per_core


_NC_CACHE = {}


def _get_nc(cap=4352):
    if cap not in _NC_CACHE:
        _NC_CACHE[cap] = _build_nc(cap)
    return _NC_CACHE[cap]


def kernel(**inputs) -> np.ndarray:
    cat = np.asarray(inputs["cat_idx"]).astype(np.int64)
    counts = np.bincount(cat, minlength=N_CORES)
    cap = max(256, int(-(-counts.max() // 256) * 256))
    maps, rows_per_core = _pack_inputs(**inputs, cap=cap)
    nc = _get_nc(cap)
    res = bass_utils.run_bass_kernel_spmd(nc, maps, core_ids=list(range(N_CORES)))
    latent = np.zeros((B, LAT), np.float32)
    for k, r in enumerate(res.results):
        rows = rows_per_core[k]
        latent[rows] = r["out"][:, :len(rows)].T
    return latent


# revision 7
# speedup vs baseline: 2.8313x; 1.1116x over previous
"""Trainium2 Bass kernel for nn_CategorySpecificInitNet (moe_routing).

kernel(**inputs) takes the FULL unsharded inputs (keys as in
reference.setup_inputs()) and returns the FULL [B, 128] float32 output.

Strategy — expert-parallel, per the spec sharding hint's dispatch-by-category
alternative:
  - the host sharding layer dispatches rows to cores by category (the
    "all-to-all dispatch by category" of expert-parallel, realized where
    all sharding happens in this harness): rows are stably sorted by
    cat_idx and core k receives category k's rows, zero-padded to a
    static per-core capacity (max category count rounded up to the
    512-row tile size);
  - every core runs the shared encoder plus exactly ONE decoder (its
    category's), so no routing, masking, or gather happens per row —
    the decoder FLOPs drop 8x vs computing all decoders densely;
  - all compute is feature-major [features(partitions), rows(free)], so
    no transposes are ever needed on device (the host passes features
    pre-transposed); outputs come back [128, cap] and the host
    inverse-permutes rows during unsharding.
  - per-core row tiles of 512; the decoder stages are software-pipelined
    one tile behind the encoder so the PE never waits on ACT/DVE
    relu latency.

Matmuls run in float32r (fp32 storage, full PE rate at N=512, ~tf32-grade
multiply precision on HW — measured ~3e-4 max rel error vs the fp32
reference, 17x better than bf16 at the same PE throughput).
"""
import sys

for _p in ("/opt/trn_rl_repo",):
    if _p not in sys.path:
        sys.path.append(_p)

import numpy as np

import concourse.bass as bass
import concourse.bacc as bacc
import concourse.mybir as mybir
import concourse.tile as tile
from concourse import bass_utils

FR = mybir.dt.float32r
F32 = mybir.dt.float32
Alu = mybir.AluOpType
ActF = mybir.ActivationFunctionType

B, C, H1, H2, HO = 32768, 768, 512, 256, 256
DH, LAT, K = 256, 128, 8
N_CORES = 8
TILE = 512
# bias_all columns: be1[4] be2[2] bf[2](=Wd1^T be3 + bd1) bd2[2] bd3[1]
OB1, OB2, OD1, OD2, OD3 = 0, 4, 6, 8, 10
NBIAS = 11


def _build_nc(cap, tile_n=512, ps_w_bufs=6, dp_bufs=3, ps_o_bufs=2):
    assert cap % 256 == 0
    tiles = [tile_n] * (cap // tile_n)
    if cap % tile_n:
        tiles.append(cap % tile_n)
    offs = [sum(tiles[:i]) for i in range(len(tiles))]
    nt = len(tiles)
    nc = bacc.Bacc(name="catnet_ep")

    fT = nc.dram_tensor("fT", (C, cap), FR, kind="ExternalInput")
    we1 = nc.dram_tensor("we1", (C, H1), FR, kind="ExternalInput")
    we2 = nc.dram_tensor("we2", (H1, H2), FR, kind="ExternalInput")
    we3 = nc.dram_tensor("we3", (H2, HO), FR, kind="ExternalInput")
    wd1 = nc.dram_tensor("wd1", (HO, DH), FR, kind="ExternalInput")
    wd2 = nc.dram_tensor("wd2", (DH, DH), FR, kind="ExternalInput")
    wd3 = nc.dram_tensor("wd3", (DH, LAT), FR, kind="ExternalInput")
    bias_all = nc.dram_tensor("bias_all", (128, NBIAS), F32, kind="ExternalInput")
    out = nc.dram_tensor("out", (LAT, cap), F32, kind="ExternalOutput")

    nC, nH1, nH2, nHO, nDH = C // 128, H1 // 128, H2 // 128, HO // 128, DH // 128

    with tile.TileContext(nc) as tc:
        with (
            tc.tile_pool(name="wp", bufs=1) as wp,
            tc.tile_pool(name="fp", bufs=2) as fp,
            tc.tile_pool(name="ap", bufs=3) as ap,
            tc.tile_pool(name="dp", bufs=dp_bufs) as dp,
            tc.tile_pool(name="ps_w", bufs=ps_w_bufs, space="PSUM") as ps_w,
            tc.tile_pool(name="ps_o", bufs=ps_o_bufs, space="PSUM") as ps_o,
        ):
            bias_t = wp.tile([128, NBIAS], F32, tag="bias")
            nc.gpsimd.dma_start(bias_t[:], bias_all[:])

            # we1 split per chunk: the first L1 matmuls start as soon as
            # their contraction chunk lands instead of after the whole tensor
            we1_t = wp.tile([128, nC, H1], FR, tag="we1")
            for c in range(nC):
                nc.gpsimd.dma_start(we1_t[:, c, :], we1[c * 128:(c + 1) * 128, :])
            we2_t = wp.tile([128, nH1, H2], FR, tag="we2")
            nc.gpsimd.dma_start(we2_t[:], we2.rearrange("(c p) h -> p c h", p=128))
            we3_t = wp.tile([128, nH2, HO], FR, tag="we3")
            nc.gpsimd.dma_start(we3_t[:], we3.rearrange("(c p) h -> p c h", p=128))
            wd1_t = wp.tile([128, nHO, DH], FR, tag="wd1")
            nc.gpsimd.dma_start(wd1_t[:], wd1.rearrange("(c p) d -> p c d", p=128))
            wd2_t = wp.tile([128, nDH, DH], FR, tag="wd2")
            nc.gpsimd.dma_start(wd2_t[:], wd2.rearrange("(c p) d -> p c d", p=128))
            wd3_t = wp.tile([128, nDH, LAT], FR, tag="wd3")
            nc.gpsimd.dma_start(wd3_t[:], wd3.rearrange("(c p) d -> p c d", p=128))

            def emit_enc(t):
                tn = tiles[t]
                sl = slice(offs[t], offs[t] + tn)
                ftb_fl = fp.tile([128, nC, tile_n], FR, tag="ft")
                ftb = ftb_fl[:, :, :tn]
                if t == 0:
                    for c in range(nC):
                        nc.sync.dma_start(ftb[:, c, :],
                                          fT[c * 128:(c + 1) * 128, sl])
                else:
                    # alternate queues so consecutive feature tiles stream in
                    # parallel instead of serializing on one HWDGE queue
                    eng = nc.sync if t % 2 == 0 else nc.gpsimd
                    eng.dma_start(
                        ftb[:], fT.rearrange("(c p) b -> p c b", p=128)[:, :, sl])
                pwsl = slice(0, tn)
                a1 = []
                for m in range(nH1):
                    pw = ps_w.tile([128, tile_n], F32, tag="pw")[:, :tn]
                    for c in range(nC):
                        nc.tensor.matmul(pw[:], we1_t[:, c, bass.ts(m, 128)],
                                         ftb[:, c, :],
                                         start=(c == 0), stop=(c == nC - 1))
                    x = ap.tile([128, tile_n], FR, tag=f"a1_{m}")[:, :tn]
                    nc.scalar.activation(x[:], pw[:], ActF.Relu,
                                         bias=bias_t[:, OB1 + m:OB1 + m + 1])
                    a1.append(x)
                a2 = []
                for m in range(nH2):
                    pw = ps_w.tile([128, tile_n], F32, tag="pw")[:, :tn]
                    for c in range(nH1):
                        nc.tensor.matmul(pw[:], we2_t[:, c, bass.ts(m, 128)], a1[c][:],
                                         start=(c == 0), stop=(c == nH1 - 1))
                    x = ap.tile([128, tile_n], FR, tag=f"a2_{m}")[:, :tn]
                    nc.scalar.activation(x[:], pw[:], ActF.Relu,
                                         bias=bias_t[:, OB2 + m:OB2 + m + 1])
                    a2.append(x)
                h = []
                for m in range(nHO):
                    pw = ps_w.tile([128, tile_n], F32, tag="pw")[:, :tn]
                    for c in range(nH2):
                        nc.tensor.matmul(pw[:], we3_t[:, c, bass.ts(m, 128)], a2[c][:],
                                         start=(c == 0), stop=(c == nH2 - 1))
                    x = ap.tile([128, tile_n], FR, tag=f"h_{m}")[:, :tn]
                    nc.scalar.activation(x[:], pw[:], ActF.Identity,
                                         bias=bias_t[:, OB3 + m:OB3 + m + 1])
                    h.append(x)
                return h

            def emit_d1(t, h):
                tn = tiles[t]
                d1 = []
                for m in range(nDH):
                    pw = ps_w.tile([128, tile_n], F32, tag="pw")[:, :tn]
                    for c in range(nHO):
                        nc.tensor.matmul(pw[:], wd1_t[:, c, bass.ts(m, 128)], h[c][:],
                                         start=(c == 0), stop=(c == nHO - 1))
                    x = dp.tile([128, tile_n], FR, tag=f"d1_{m}")[:, :tn]
                    nc.scalar.activation(x[:], pw[:], ActF.Relu,
                                         bias=bias_t[:, OD1 + m:OD1 + m + 1])
                    d1.append(x)
                return d1

            def emit_d2_d3_store(t, d1):
                tn = tiles[t]
                d2 = []
                for m in range(nDH):
                    pw = ps_w.tile([128, tile_n], F32, tag="pw")[:, :tn]
                    for c in range(nDH):
                        nc.tensor.matmul(pw[:], wd2_t[:, c, bass.ts(m, 128)], d1[c][:],
                                         start=(c == 0), stop=(c == nDH - 1))
                    x = dp.tile([128, tile_n], FR, tag=f"d2_{m}")[:, :tn]
                    bb = bias_t[:, OD2 + m:OD2 + m + 1]
                    if m % 2 == 0:
                        nc.vector.tensor_scalar(x[:], pw[:], bb, 0.0, Alu.add, Alu.max)
                    else:
                        nc.scalar.activation(x[:], pw[:], ActF.Relu, bias=bb)
                    d2.append(x)
                po = ps_o.tile([128, tile_n], F32, tag="out")[:, :tn]
                for c in range(nDH):
                    nc.tensor.matmul(po[:], wd3_t[:, c, :], d2[c][:],
                                     start=(c == 0), stop=(c == nDH - 1))
                osb = ap.tile([128, tile_n], F32, tag="osb")[:, :tn]
                nc.scalar.activation(osb[:], po[:], ActF.Identity,
                                     bias=bias_t[:, OD3:OD3 + 1])
                nc.gpsimd.dma_start(out[:, offs[t]:offs[t] + tn], osb[:])

            # decoder runs up to two tiles behind the encoder: PE order per
            # step is [enc t][d2/d3 t-2][d1 t-1], maximizing matmul slack to
            # hide ACT latency
            import os
            deep = os.environ.get("EP_DEEP", "0") == "1"
            if deep:
                ph = None   # (t, h) awaiting d1
                pd = None   # (t, d1) awaiting d2/d3
                for t in range(nt):
                    h = emit_enc(t)
                    if pd is not None:
                        emit_d2_d3_store(pd[0], pd[1])
                        pd = None
                    if ph is not None:
                        pd = (ph[0], emit_d1(ph[0], ph[1]))
                    ph = (t, h)
                pd2 = (ph[0], emit_d1(ph[0], ph[1]))
                if pd is not None:
                    emit_d2_d3_store(pd[0], pd[1])
                emit_d2_d3_store(pd2[0], pd2[1])
            else:
                pend = None
                for t in range(nt):
                    h = emit_enc(t)
                    if pend is not None:
                        emit_d2_d3_store(pend[0], pend[1])
                    d1 = emit_d1(t, h)
                    pend = (t, d1)
                emit_d2_d3_store(pend[0], pend[1])

    nc.finalize()
    return nc


def _pack_inputs(features, We1, be1, We2, be2, We3, be3,
                 Wd1, bd1, Wd2, bd2, Wd3, bd3, cat_idx, cap):
    """Dispatch rows to cores by category (expert-parallel sharding)."""
    features = np.asarray(features, np.float32)
    cat = np.asarray(cat_idx).astype(np.int64)
    order = np.argsort(cat, kind="stable")
    counts = np.bincount(cat, minlength=N_CORES)
    starts = np.zeros(N_CORES + 1, np.int64)
    np.cumsum(counts, out=starts[1:])

    def chunkcols(b):
        b = np.asarray(b, np.float32).reshape(-1)
        return b.reshape(-1, 128).T

    enc = dict(
        we1=np.asarray(We1, np.float32), we2=np.asarray(We2, np.float32),
    )
    We3f = np.asarray(We3, np.float32)
    be3f = np.asarray(be3, np.float32)
    maps, rows_per_core = [], []
    for k in range(N_CORES):
        rows = order[starts[k]:starts[k + 1]]
        rows_per_core.append(rows)
        f = np.zeros((cap, C), np.float32)
        f[:len(rows)] = features[rows]
        bias_all = np.zeros((128, NBIAS), np.float32)
        bias_all[:, OB1:OB1 + 4] = chunkcols(be1)
        bias_all[:, OB2:OB2 + 2] = chunkcols(be2)
        wd1k = np.asarray(Wd1, np.float32)[k]
        bias_all[:, OD1:OD1 + 2] = chunkcols(
            wd1k.T @ be3f + np.asarray(bd1, np.float32)[k])
        bias_all[:, OD2:OD2 + 2] = chunkcols(np.asarray(bd2, np.float32)[k])
        bias_all[:, OD3:OD3 + 1] = chunkcols(np.asarray(bd3, np.float32)[k])
        m = dict(enc)
        m["fT"] = np.ascontiguousarray(f.T)
        m["wd1"] = We3f @ wd1k  # encoder L3 folded into decoder layer 1
        m["wd2"] = np.asarray(Wd2, np.float32)[k]
        m["wd3"] = np.asarray(Wd3, np.float32)[k]
        m["bias_all"] = bias_all
        maps.append(m)
    return maps, rows_per_core


_NC_CACHE = {}


def _get_nc(cap=4352):
    if cap not in _NC_CACHE:
        _NC_CACHE[cap] = _build_nc(cap)
    return _NC_CACHE[cap]


def kernel(**inputs) -> np.ndarray:
    cat = np.asarray(inputs["cat_idx"]).astype(np.int64)
    counts = np.bincount(cat, minlength=N_CORES)
    cap = max(256, int(-(-counts.max() // 256) * 256))
    maps, rows_per_core = _pack_inputs(**inputs, cap=cap)
    nc = _get_nc(cap)
    res = bass_utils.run_bass_kernel_spmd(nc, maps, core_ids=list(range(N_CORES)))
    latent = np.zeros((B, LAT), np.float32)
    for k, r in enumerate(res.results):
        rows = rows_per_core[k]
        latent[rows] = r["out"][:, :len(rows)].T
    return latent


# revision 8
# speedup vs baseline: 2.8481x; 1.0059x over previous
"""Trainium2 Bass kernel for nn_CategorySpecificInitNet (moe_routing).

kernel(**inputs) takes the FULL unsharded inputs (keys as in
reference.setup_inputs()) and returns the FULL [B, 128] float32 output.

Strategy — expert-parallel, per the spec sharding hint's dispatch-by-category
alternative:
  - the host sharding layer dispatches rows to cores by category (the
    "all-to-all dispatch by category" of expert-parallel, realized where
    all sharding happens in this harness): rows are stably sorted by
    cat_idx and core k receives category k's rows, zero-padded to a
    static per-core capacity (max category count rounded up to the
    512-row tile size);
  - every core runs the shared encoder plus exactly ONE decoder (its
    category's), so no routing, masking, or gather happens per row —
    the decoder FLOPs drop 8x vs computing all decoders densely;
  - the encoder's linear third layer is constant-folded into the
    decoder's first layer on the host (W_f = We3 @ Wd1_k,
    b_f = Wd1_k^T be3 + bd1_k — exact algebra, ~0.1% of the FLOPs),
    removing one full matmul stage from the device;
  - all compute is feature-major [features(partitions), rows(free)], so
    no transposes are ever needed on device (the host passes features
    pre-transposed); outputs come back [128, cap] and the host
    inverse-permutes rows during unsharding.
  - per-core row tiles of 512; the decoder stages are software-pipelined
    one tile behind the encoder so the PE never waits on ACT/DVE
    relu latency.

Matmuls run in float32r (fp32 storage, full PE rate at N=512, ~tf32-grade
multiply precision on HW — measured ~3e-4 max rel error vs the fp32
reference, 17x better than bf16 at the same PE throughput).
"""
import sys

for _p in ("/opt/trn_rl_repo",):
    if _p not in sys.path:
        sys.path.append(_p)

import numpy as np

import concourse.bass as bass
import concourse.bacc as bacc
import concourse.mybir as mybir
import concourse.tile as tile
from concourse import bass_utils

FR = mybir.dt.float32r
F32 = mybir.dt.float32
Alu = mybir.AluOpType
ActF = mybir.ActivationFunctionType

B, C, H1, H2, HO = 32768, 768, 512, 256, 256
DH, LAT, K = 256, 128, 8
N_CORES = 8
TILE = 512
# bias_all columns: be1[4] be2[2] bf[2](=Wd1^T be3 + bd1) bd2[2] bd3[1]
OB1, OB2, OD1, OD2, OD3 = 0, 4, 6, 8, 10
NBIAS = 11


def _build_nc(cap, tile_n=512, ps_w_bufs=6, dp_bufs=3, ps_o_bufs=2):
    assert cap % 256 == 0
    tiles = [tile_n] * (cap // tile_n)
    if cap % tile_n:
        tiles.append(cap % tile_n)
    offs = [sum(tiles[:i]) for i in range(len(tiles))]
    nt = len(tiles)
    nc = bacc.Bacc(name="catnet_ep")

    fT = nc.dram_tensor("fT", (C, cap), FR, kind="ExternalInput")
    we1 = nc.dram_tensor("we1", (C, H1), FR, kind="ExternalInput")
    we2 = nc.dram_tensor("we2", (H1, H2), FR, kind="ExternalInput")
    we3 = nc.dram_tensor("we3", (H2, HO), FR, kind="ExternalInput")
    wd1 = nc.dram_tensor("wd1", (HO, DH), FR, kind="ExternalInput")
    wd2 = nc.dram_tensor("wd2", (DH, DH), FR, kind="ExternalInput")
    wd3 = nc.dram_tensor("wd3", (DH, LAT), FR, kind="ExternalInput")
    bias_all = nc.dram_tensor("bias_all", (128, NBIAS), F32, kind="ExternalInput")
    out = nc.dram_tensor("out", (LAT, cap), F32, kind="ExternalOutput")

    nC, nH1, nH2, nHO, nDH = C // 128, H1 // 128, H2 // 128, HO // 128, DH // 128

    with tile.TileContext(nc) as tc:
        with (
            tc.tile_pool(name="wp", bufs=1) as wp,
            tc.tile_pool(name="fp", bufs=2) as fp,
            tc.tile_pool(name="ap", bufs=3) as ap,
            tc.tile_pool(name="dp", bufs=dp_bufs) as dp,
            tc.tile_pool(name="ps_w", bufs=ps_w_bufs, space="PSUM") as ps_w,
            tc.tile_pool(name="ps_o", bufs=ps_o_bufs, space="PSUM") as ps_o,
        ):
            bias_t = wp.tile([128, NBIAS], F32, tag="bias")
            nc.gpsimd.dma_start(bias_t[:], bias_all[:])

            # we1 split per chunk: the first L1 matmuls start as soon as
            # their contraction chunk lands instead of after the whole tensor
            we1_t = wp.tile([128, nC, H1], FR, tag="we1")
            for c in range(nC):
                nc.gpsimd.dma_start(we1_t[:, c, :], we1[c * 128:(c + 1) * 128, :])
            we2_t = wp.tile([128, nH1, H2], FR, tag="we2")
            nc.gpsimd.dma_start(we2_t[:], we2.rearrange("(c p) h -> p c h", p=128))
            we3_t = wp.tile([128, nH2, HO], FR, tag="we3")
            nc.gpsimd.dma_start(we3_t[:], we3.rearrange("(c p) h -> p c h", p=128))
            wd1_t = wp.tile([128, nHO, DH], FR, tag="wd1")
            nc.gpsimd.dma_start(wd1_t[:], wd1.rearrange("(c p) d -> p c d", p=128))
            wd2_t = wp.tile([128, nDH, DH], FR, tag="wd2")
            nc.gpsimd.dma_start(wd2_t[:], wd2.rearrange("(c p) d -> p c d", p=128))
            wd3_t = wp.tile([128, nDH, LAT], FR, tag="wd3")
            nc.gpsimd.dma_start(wd3_t[:], wd3.rearrange("(c p) d -> p c d", p=128))

            def emit_enc(t):
                tn = tiles[t]
                sl = slice(offs[t], offs[t] + tn)
                ftb_fl = fp.tile([128, nC, tile_n], FR, tag="ft")
                ftb = ftb_fl[:, :, :tn]
                if t == 0:
                    for c in range(nC):
                        nc.sync.dma_start(ftb[:, c, :],
                                          fT[c * 128:(c + 1) * 128, sl])
                else:
                    # alternate queues so consecutive feature tiles stream in
                    # parallel instead of serializing on one HWDGE queue
                    eng = nc.sync if t % 2 == 0 else nc.gpsimd
                    eng.dma_start(
                        ftb[:], fT.rearrange("(c p) b -> p c b", p=128)[:, :, sl])
                pwsl = slice(0, tn)
                a1 = []
                for m in range(nH1):
                    pw = ps_w.tile([128, tile_n], F32, tag="pw")[:, :tn]
                    for c in range(nC):
                        nc.tensor.matmul(pw[:], we1_t[:, c, bass.ts(m, 128)],
                                         ftb[:, c, :],
                                         start=(c == 0), stop=(c == nC - 1))
                    x = ap.tile([128, tile_n], FR, tag=f"a1_{m}")[:, :tn]
                    nc.scalar.activation(x[:], pw[:], ActF.Relu,
                                         bias=bias_t[:, OB1 + m:OB1 + m + 1])
                    a1.append(x)
                a2 = []
                for m in range(nH2):
                    pw = ps_w.tile([128, tile_n], F32, tag="pw")[:, :tn]
                    for c in range(nH1):
                        nc.tensor.matmul(pw[:], we2_t[:, c, bass.ts(m, 128)], a1[c][:],
                                         start=(c == 0), stop=(c == nH1 - 1))
                    x = ap.tile([128, tile_n], FR, tag=f"a2_{m}")[:, :tn]
                    nc.scalar.activation(x[:], pw[:], ActF.Relu,
                                         bias=bias_t[:, OB2 + m:OB2 + m + 1])
                    a2.append(x)
                h = []
                for m in range(nHO):
                    pw = ps_w.tile([128, tile_n], F32, tag="pw")[:, :tn]
                    for c in range(nH2):
                        nc.tensor.matmul(pw[:], we3_t[:, c, bass.ts(m, 128)], a2[c][:],
                                         start=(c == 0), stop=(c == nH2 - 1))
                    x = ap.tile([128, tile_n], FR, tag=f"h_{m}")[:, :tn]
                    nc.scalar.activation(x[:], pw[:], ActF.Identity,
                                         bias=bias_t[:, OB3 + m:OB3 + m + 1])
                    h.append(x)
                return h

            def emit_d1(t, h):
                tn = tiles[t]
                d1 = []
                for m in range(nDH):
                    pw = ps_w.tile([128, tile_n], F32, tag="pw")[:, :tn]
                    for c in range(nHO):
                        nc.tensor.matmul(pw[:], wd1_t[:, c, bass.ts(m, 128)], h[c][:],
                                         start=(c == 0), stop=(c == nHO - 1))
                    x = dp.tile([128, tile_n], FR, tag=f"d1_{m}")[:, :tn]
                    nc.scalar.activation(x[:], pw[:], ActF.Relu,
                                         bias=bias_t[:, OD1 + m:OD1 + m + 1])
                    d1.append(x)
                return d1

            def emit_d2_d3_store(t, d1):
                tn = tiles[t]
                d2 = []
                for m in range(nDH):
                    pw = ps_w.tile([128, tile_n], F32, tag="pw")[:, :tn]
                    for c in range(nDH):
                        nc.tensor.matmul(pw[:], wd2_t[:, c, bass.ts(m, 128)], d1[c][:],
                                         start=(c == 0), stop=(c == nDH - 1))
                    x = dp.tile([128, tile_n], FR, tag=f"d2_{m}")[:, :tn]
                    bb = bias_t[:, OD2 + m:OD2 + m + 1]
                    if m % 2 == 0:
                        nc.vector.tensor_scalar(x[:], pw[:], bb, 0.0, Alu.add, Alu.max)
                    else:
                        nc.scalar.activation(x[:], pw[:], ActF.Relu, bias=bb)
                    d2.append(x)
                po = ps_o.tile([128, tile_n], F32, tag="out")[:, :tn]
                for c in range(nDH):
                    nc.tensor.matmul(po[:], wd3_t[:, c, :], d2[c][:],
                                     start=(c == 0), stop=(c == nDH - 1))
                osb = ap.tile([128, tile_n], F32, tag="osb")[:, :tn]
                nc.scalar.activation(osb[:], po[:], ActF.Identity,
                                     bias=bias_t[:, OD3:OD3 + 1])
                nc.gpsimd.dma_start(out[:, offs[t]:offs[t] + tn], osb[:])

            # decoder runs up to two tiles behind the encoder: PE order per
            # step is [enc t][d2/d3 t-2][d1 t-1], maximizing matmul slack to
            # hide ACT latency
            import os
            deep = os.environ.get("EP_DEEP", "0") == "1"
            if deep:
                ph = None   # (t, h) awaiting d1
                pd = None   # (t, d1) awaiting d2/d3
                for t in range(nt):
                    h = emit_enc(t)
                    if pd is not None:
                        emit_d2_d3_store(pd[0], pd[1])
                        pd = None
                    if ph is not None:
                        pd = (ph[0], emit_d1(ph[0], ph[1]))
                    ph = (t, h)
                pd2 = (ph[0], emit_d1(ph[0], ph[1]))
                if pd is not None:
                    emit_d2_d3_store(pd[0], pd[1])
                emit_d2_d3_store(pd2[0], pd2[1])
            else:
                pend = None
                for t in range(nt):
                    h = emit_enc(t)
                    if pend is not None:
                        emit_d2_d3_store(pend[0], pend[1])
                    d1 = emit_d1(t, h)
                    pend = (t, d1)
                emit_d2_d3_store(pend[0], pend[1])

    nc.finalize()
    return nc


def _pack_inputs(features, We1, be1, We2, be2, We3, be3,
                 Wd1, bd1, Wd2, bd2, Wd3, bd3, cat_idx, cap):
    """Dispatch rows to cores by category (expert-parallel sharding)."""
    features = np.asarray(features, np.float32)
    cat = np.asarray(cat_idx).astype(np.int64)
    order = np.argsort(cat, kind="stable")
    counts = np.bincount(cat, minlength=N_CORES)
    starts = np.zeros(N_CORES + 1, np.int64)
    np.cumsum(counts, out=starts[1:])

    def chunkcols(b):
        b = np.asarray(b, np.float32).reshape(-1)
        return b.reshape(-1, 128).T

    enc = dict(
        we1=np.asarray(We1, np.float32), we2=np.asarray(We2, np.float32),
    )
    We3f = np.asarray(We3, np.float32)
    be3f = np.asarray(be3, np.float32)
    maps, rows_per_core = [], []
    for k in range(N_CORES):
        rows = order[starts[k]:starts[k + 1]]
        rows_per_core.append(rows)
        f = np.zeros((cap, C), np.float32)
        f[:len(rows)] = features[rows]
        bias_all = np.zeros((128, NBIAS), np.float32)
        bias_all[:, OB1:OB1 + 4] = chunkcols(be1)
        bias_all[:, OB2:OB2 + 2] = chunkcols(be2)
        wd1k = np.asarray(Wd1, np.float32)[k]
        bias_all[:, OD1:OD1 + 2] = chunkcols(
            wd1k.T @ be3f + np.asarray(bd1, np.float32)[k])
        bias_all[:, OD2:OD2 + 2] = chunkcols(np.asarray(bd2, np.float32)[k])
        bias_all[:, OD3:OD3 + 1] = chunkcols(np.asarray(bd3, np.float32)[k])
        m = dict(enc)
        m["fT"] = np.ascontiguousarray(f.T)
        m["wd1"] = We3f @ wd1k  # encoder L3 folded into decoder layer 1
        m["wd2"] = np.asarray(Wd2, np.float32)[k]
        m["wd3"] = np.asarray(Wd3, np.float32)[k]
        m["bias_all"] = bias_all
        maps.append(m)
    return maps, rows_per_core


_NC_CACHE = {}


def _get_nc(cap=4352):
    if cap not in _NC_CACHE:
        _NC_CACHE[cap] = _build_nc(cap)
    return _NC_CACHE[cap]


def kernel(**inputs) -> np.ndarray:
    cat = np.asarray(inputs["cat_idx"]).astype(np.int64)
    counts = np.bincount(cat, minlength=N_CORES)
    cap = max(256, int(-(-counts.max() // 256) * 256))
    maps, rows_per_core = _pack_inputs(**inputs, cap=cap)
    nc = _get_nc(cap)
    res = bass_utils.run_bass_kernel_spmd(nc, maps, core_ids=list(range(N_CORES)))
    latent = np.zeros((B, LAT), np.float32)
    for k, r in enumerate(res.results):
        rows = rows_per_core[k]
        latent[rows] = r["out"][:, :len(rows)].T
    return latent
